# revision 1
# baseline (speedup 1.0000x reference)
"""GIN-style 5-layer GNN message passing on 8 Trainium2 NeuronCores.

Strategy v4 (1D node-parallel, tensor-engine aggregation):
  - Nodes partitioned contiguously across 8 cores (12500 each, padded to
    12544 = 98*128). Edges owned by their dst core. The bottleneck is
    GPSIMD/SWDGE per-edge gather-descriptor generation (~8 ns/idx), so
    everything else is arranged to hide under it.
  - Per layer: two AllGathers (shard halves, bf16) build the full node
    table in DRAM; each half's ncfw work overlaps the previous/current
    gather passes.  dma_gather fetches h[src] per edge (256B rows) in
    dst-tile-sorted order, 2048 idxs per call, window-major passes.
  - Aggregation runs on the tensor engine, not the DMA scatter path:
    per 128-edge group (all edges of one dst tile), a one-hot fp8
    selection matrix contracts the gathered bf16 messages into a PSUM
    accumulator (out[dst, feat] += sel^T @ msg), window partials summed
    into an SBUF accumulator by the vector engine.  The self-loop term
    is the bank-filling first matmul (identity x h_own slab,
    start=True); edge embeddings fold into a counts matmul
    (counts^T @ etab).
  - Per-(tile, window) gather slots are capped at 512 (the cell mean is
    ~510); overflow edges (~3k/layer) go through a small classic
    dma_gather + dma_scatter_add pipeline into a DRAM side-accumulator,
    merged back by one identity matmul per tile.  Scatter sub-calls
    keep dst rows distinct (the DMA's RMW races on duplicates) and pad
    with trash rows >= NPAD (negative indices fault).
  - GIN MLP (D->2D->relu->D) + BatchNorm folded into the second linear,
    bf16 weights, computed per 128-node tile on the tensor engine.
"""
import sys
import numpy as np

sys.path.insert(0, "/opt/trn_rl_repo")

import ml_dtypes
import concourse.bass as bass
import concourse.bacc as bacc
import concourse.tile as tile
import concourse.masks as masks
from concourse import mybir
from concourse.bass_utils import run_bass_kernel_spmd


class CFG:
    DEBUG_TAPS = False
    TAPS = ()
    SPLIT_AG = True
    N = 100000          # total nodes
    D = 128             # feature dim
    L = 5               # layers
    NCORE = 8
    NOWN = 12500        # nodes per core
    NPAD = 12544        # padded nodes per core (98 * 128)
    NBLK = 4            # gather source windows (int16 idx limit)
    TPB = 4             # dst tiles per block
    CAP = 512           # slot capacity per (tile, window) cell; 4 groups
    # overflow sub-call capacities per window (scatter-add path);
    # each sub-call holds edges with distinct dst nodes
    OCAPS = (1024, 512, 256, 128, 128)
    EPS = 1e-5

    @property
    def OWIN(self):     # overflow slots per window
        return sum(self.OCAPS)

    @property
    def WIN(self):      # rows per gather window in h_full space
        return 2 * self.NPAD

    @property
    def NTILE(self):
        return self.NPAD // 128

    @property
    def HFULL(self):
        return self.NCORE * self.NPAD

    @property
    def GPC(self):      # groups per cell
        return self.CAP // 128

    @property
    def blocks(self):   # list of tile-index lists
        t = list(range(self.NTILE))
        return [t[i:i + self.TPB] for i in range(0, self.NTILE, self.TPB)]

    @property
    def TOTSLOT(self):
        return self.NTILE * self.NBLK * self.CAP

    @property
    def TOTGRP(self):
        return self.TOTSLOT // 128


def _f8(a):
    return np.asarray(a, np.float32).astype(ml_dtypes.float8_e4m3fn)


def _bf(a):
    return np.asarray(a, np.float32).astype(ml_dtypes.bfloat16)


def _fold_params(cfg, x_emb, etab, w1, b1, w2, b2, gamma, beta, bn_mean, bn_var):
    """Host-side parameter folding. Returns replicated device param arrays."""
    D, L = cfg.D, cfg.L
    x_emb = np.asarray(x_emb, np.float64)
    etab = np.asarray(etab, np.float64)
    w1 = np.asarray(w1, np.float64)
    b1 = np.asarray(b1, np.float64)
    w2 = np.asarray(w2, np.float64)
    b2 = np.asarray(b2, np.float64)
    gamma = np.asarray(gamma, np.float64)
    beta = np.asarray(beta, np.float64)
    bn_mean = np.asarray(bn_mean, np.float64)
    bn_var = np.asarray(bn_var, np.float64)

    xemb6 = np.zeros((8, D), np.float64)
    xemb6[0:3] = x_emb[0:3]
    xemb6[3:6] = x_emb[120:123]

    etab9 = np.zeros((L, 16, D), np.float64)
    etab9[:, 0:9, :] = etab

    w1t = np.ascontiguousarray(np.transpose(w1, (0, 2, 1)))          # [L,D,2D]
    b1t = np.ascontiguousarray(
        b1.reshape(L, 2, D).transpose(0, 2, 1)).astype(np.float32)   # [L,D,2]

    s = gamma / np.sqrt(bn_var + cfg.EPS)          # [L, D]
    t = beta - bn_mean * s
    w2f = w2 * s[:, :, None]                       # [L, D, 2D] rows scaled
    b2f = b2 * s + t                               # [L, D]
    # stationary chunks: w2s[l, p, k, m] = w2f[l, m, k*128 + p]
    w2s = np.ascontiguousarray(
        np.transpose(w2f.reshape(L, D, 2, D), (0, 3, 2, 1)))         # [L,128,2,128]
    b2t = b2f.astype(np.float32).reshape(L, D, 1)
    return dict(xemb6=_bf(xemb6), etab9=_bf(etab9), w1t=_bf(w1t), b1t=b1t,
                w2s=_bf(w2s), b2t=b2t)


def _wrap16(a):
    """Element i -> [i % 16, i // 16], replicated to 128 partitions."""
    assert len(a) % 16 == 0
    w = a.reshape(-1, 16).T
    return np.ascontiguousarray(np.tile(w, (8, 1)))


def _cell_bases(cfg):
    """slot base for each (tile, window) cell, matching the call layout:
    for each block B, for each window w, tiles of B consecutively."""
    bases = np.zeros((cfg.NTILE, cfg.NBLK), np.int64)
    off = 0
    for blk in cfg.blocks:
        for w in range(cfg.NBLK):
            for t in blk:
                bases[t, w] = off
                off += cfg.CAP
    assert off == cfg.TOTSLOT
    return bases


def _schedule_core(cfg, src_g, dst_l):
    """Assign this core's edges (global src, local dst) to gather slots.

    Returns gidx [TOTSLOT] int16 (window-local gather idx; 0 for padding)
    and selT [128, TOTGRP, 128] fp8 one-hot matrices (zero rows for pads).
    """
    # hfull is split into two AllGather halves: A = rank-major concat of
    # each core's rows [0:HALF), B = rows [HALF:NPAD).  Window w in {0,1}
    # addresses A, {2,3} addresses B.
    HALF = cfg.NPAD // 2
    q = src_g // cfg.NOWN
    local = src_g - q * cfg.NOWN
    if cfg.SPLIT_AG:
        in_a = local < HALF
        halfrow = np.where(in_a, q * HALF + local, q * HALF + local - HALF)
        w = halfrow // cfg.WIN + np.where(in_a, 0, 2)
        widx = (halfrow % cfg.WIN).astype(np.int64)
    else:
        src_row = q * cfg.NPAD + local
        w = src_row // cfg.WIN
        widx = (src_row % cfg.WIN).astype(np.int64)
    assert widx.max() < 2 ** 15
    t = dst_l // 128
    j = dst_l % 128

    cell = t * cfg.NBLK + w
    order = np.argsort(cell, kind="stable")
    cell_s = cell[order]
    counts = np.bincount(cell_s, minlength=cfg.NTILE * cfg.NBLK)
    bases = _cell_bases(cfg).reshape(-1)
    cell_starts = np.zeros(cfg.NTILE * cfg.NBLK + 1, np.int64)
    np.cumsum(counts, out=cell_starts[1:])
    pos_in_cell = np.arange(len(cell_s)) - cell_starts[cell_s]
    in_main = pos_in_cell < cfg.CAP
    slot = bases[cell_s] + pos_in_cell

    gidx = np.zeros(cfg.TOTSLOT, np.int16)
    gidx[slot[in_main]] = widx[order][in_main].astype(np.int16)
    selT = np.zeros((128, cfg.TOTGRP, 128), ml_dtypes.float8_e4m3fn)
    selT[slot[in_main] % 128, slot[in_main] // 128,
         j[order][in_main]] = 1.0

    # ---- overflow edges -> per-window scatter-add sub-calls ----
    # slot layout: [window][sub-call][cap]; within each sub-call dst
    # nodes are distinct.  Unused slots scatter row 0 of the window into
    # trash rows >= NPAD (negative "ignored" indices fault on this HW).
    nslot = cfg.NBLK * cfg.OWIN
    gidxo = np.zeros(nslot, np.int16)
    sidxo = (cfg.NPAD + np.arange(nslot) % 1024).astype(np.int16)
    ov = ~in_main
    ov_w = (cell_s % cfg.NBLK)[ov]
    ov_widx = widx[order][ov]
    ov_dst = dst_l[order][ov]
    for wv in range(cfg.NBLK):
        m = ov_w == wv
        dsts = ov_dst[m]
        wis = ov_widx[m]
        fill = [0] * len(cfg.OCAPS)
        used = [set() for _ in cfg.OCAPS]
        woff = wv * cfg.OWIN
        for dd, wi in zip(dsts.tolist(), wis.tolist()):
            for k, cap in enumerate(cfg.OCAPS):
                if fill[k] < cap and dd not in used[k]:
                    off = woff + sum(cfg.OCAPS[:k]) + fill[k]
                    gidxo[off] = wi
                    sidxo[off] = dd
                    used[k].add(dd)
                    fill[k] += 1
                    break
            else:
                raise AssertionError("overflow sub-calls exhausted")
    # gather pads must be valid indices (interior positions); the
    # matching scatter slots stay -1 so their values are never used
    gpad = gidxo < 0
    gidxo[gpad] = 0
    return gidx, selT, gidxo, sidxo


def _prepare_inputs(cfg, x, edge_index, edge_attr):
    """Per-core index/feature-side host prep."""
    x = np.asarray(x)
    src = np.asarray(edge_index[0], np.int64)
    dst = np.asarray(edge_index[1], np.int64)
    eb = np.asarray(edge_attr[:, 0], np.int64)
    ed = np.asarray(edge_attr[:, 1], np.int64)

    per_core = []
    owner = dst // cfg.NOWN
    for r in range(cfg.NCORE):
        m = owner == r
        dst_l = dst[m] - r * cfg.NOWN
        gidx, selT, gidxo, sidxo = _schedule_core(cfg, src[m], dst_l)

        # counts[j, n]: incoming (incl self-loop) bond-type/direction counts
        countsT = np.zeros((16, cfg.NPAD), np.float32)
        np.add.at(countsT, (eb[m], dst_l), 1.0)
        np.add.at(countsT, (6 + ed[m], dst_l), 1.0)
        loc = np.arange(cfg.NOWN)
        countsT[4, loc] += 1.0   # self-loop bond type 4
        countsT[6, loc] += 1.0   # self-loop direction 0

        xohT = np.zeros((8, cfg.NPAD), np.float32)
        xl = np.asarray(x[r * cfg.NOWN:(r + 1) * cfg.NOWN], np.int64)
        xohT[xl[:, 0], loc] = 1.0
        xohT[3 + xl[:, 1], loc] += 1.0

        per_core.append(dict(
            gidx=_wrap16(gidx), selT=selT,
            gidxo=_wrap16(gidxo), sidxo=_wrap16(sidxo),
            countsT=_bf(countsT), xohT=_bf(xohT),
        ))
    return per_core


def _build_program(cfg):
    nc = bacc.Bacc(None, target_bir_lowering=False, debug=True)
    f32, bf16, i16 = mybir.dt.float32, mybir.dt.bfloat16, mybir.dt.int16
    fp8 = mybir.dt.float8e4
    D, L = cfg.D, cfg.L
    GPC = cfg.GPC

    # I/O
    gidx_in = nc.dram_tensor("gidx", [128, cfg.TOTSLOT // 16], i16,
                             kind="ExternalInput")
    gidxo_in = nc.dram_tensor("gidxo", [128, cfg.NBLK * cfg.OWIN // 16], i16,
                              kind="ExternalInput")
    sidxo_in = nc.dram_tensor("sidxo", [128, cfg.NBLK * cfg.OWIN // 16], i16,
                              kind="ExternalInput")
    selT_in = nc.dram_tensor("selT", [128, cfg.TOTGRP, 128], fp8,
                             kind="ExternalInput")
    countsT_in = nc.dram_tensor("countsT", [16, cfg.NPAD], bf16,
                                kind="ExternalInput")
    xohT_in = nc.dram_tensor("xohT", [8, cfg.NPAD], bf16, kind="ExternalInput")
    xemb6_in = nc.dram_tensor("xemb6", [8, D], bf16, kind="ExternalInput")
    etab9_in = nc.dram_tensor("etab9", [L, 16, D], bf16, kind="ExternalInput")
    w1t_in = nc.dram_tensor("w1t", [L, D, 2 * D], bf16, kind="ExternalInput")
    b1t_in = nc.dram_tensor("b1t", [L, D, 2], f32, kind="ExternalInput")
    w2s_in = nc.dram_tensor("w2s", [L, 128, 2, 128], bf16, kind="ExternalInput")
    b2t_in = nc.dram_tensor("b2t", [L, D, 1], f32, kind="ExternalInput")
    out_ext = nc.dram_tensor("out", [cfg.NPAD, D], f32, kind="ExternalOutput")
    if cfg.DEBUG_TAPS:
        dbg_hown0 = nc.dram_tensor("dbg_hown0", [cfg.NPAD, D], f32,
                                   kind="ExternalOutput")
        dbg_hfA = nc.dram_tensor("dbg_hfA", [cfg.NCORE * (cfg.NPAD // 2), D],
                                 f32, kind="ExternalOutput")
        dbg_agg = nc.dram_tensor("dbg_agg", [cfg.NPAD, D], f32,
                                 kind="ExternalOutput")
        dbg_hown1 = nc.dram_tensor("dbg_hown1", [cfg.NPAD, D], f32,
                                   kind="ExternalOutput")

    # internal DRAM (shared across layers; layers are serial)
    HALF = cfg.NPAD // 2
    if cfg.SPLIT_AG:
        hownA = nc.dram_tensor("hownA", [HALF, D], bf16)
        hownB = nc.dram_tensor("hownB", [HALF, D], bf16)
        hfullA = nc.dram_tensor("hfullA", [cfg.NCORE * HALF, D], bf16,
                                addr_space="Shared")
        hfullB = nc.dram_tensor("hfullB", [cfg.NCORE * HALF, D], bf16,
                                addr_space="Shared")
        hown_d = None
    else:
        hown_d = nc.dram_tensor("hown", [cfg.NPAD, D], bf16)
        hfull = nc.dram_tensor("hfull", [cfg.HFULL, D], bf16,
                               addr_space="Shared")
        hfullA = hfullB = None

    ovf_d = nc.dram_tensor("ovf", [cfg.NPAD + 1024, D], bf16)

    def hown_rows(t):
        # DMA destination for the h rows of tile t (128 rows)
        r0 = t * 128
        if not cfg.SPLIT_AG:
            return hown_d[r0:r0 + 128, :]
        if r0 < HALF:
            return hownA[r0:r0 + 128, :]
        return hownB[r0 - HALF:r0 - HALF + 128, :]

    relu = mybir.ActivationFunctionType.Relu

    with tile.TileContext(nc) as tc:
        with (
            tc.tile_pool(name="const", bufs=1) as const_pool,
            tc.tile_pool(name="gather", bufs=4) as gather_pool,
            tc.tile_pool(name="sel", bufs=4) as sel_pool,
            tc.tile_pool(name="mlp", bufs=3) as mlp_pool,
            tc.tile_pool(name="aggp", bufs=2, space="PSUM") as agg_pool,
            tc.tile_pool(name="psA", bufs=1, space="PSUM") as psA_pool,
            tc.tile_pool(name="psB", bufs=2, space="PSUM") as psB_pool,
        ):
            # ---- resident constants ----
            identf = const_pool.tile([128, 128], f32, tag="identf")
            masks.make_identity(nc, identf[:, :])
            identb = const_pool.tile([128, 128], bf16, tag="identb")
            nc.vector.tensor_copy(identb[:, :], identf[:, :])
            zerob = const_pool.tile([128, cfg.TPB, 128], bf16, tag="zerob")
            nc.gpsimd.memset(zerob[:, :, :], 0.0)
            agg_sb = const_pool.tile([128, cfg.NTILE, 128], f32, tag="agg_sb")
            gidx_t = const_pool.tile([128, cfg.TOTSLOT // 16], i16, tag="gidx")
            nc.sync.dma_start(gidx_t[:, :], gidx_in[:, :])
            gidxo_t = const_pool.tile([128, cfg.NBLK * cfg.OWIN // 16], i16,
                                      tag="gidxo")
            nc.sync.dma_start(gidxo_t[:, :], gidxo_in[:, :])
            sidxo_t = const_pool.tile([128, cfg.NBLK * cfg.OWIN // 16], i16,
                                      tag="sidxo")
            nc.sync.dma_start(sidxo_t[:, :], sidxo_in[:, :])
            cntT = const_pool.tile([16, cfg.NPAD], bf16, tag="cntT")
            nc.sync.dma_start(cntT[:, :], countsT_in[:, :])
            hown_sb = const_pool.tile([128, cfg.NTILE, 128], bf16, tag="hown_sb")
            xemb6 = const_pool.tile([8, D], bf16, tag="xemb6")
            nc.sync.dma_start(xemb6[:, :], xemb6_in[:, :])
            etab9 = [const_pool.tile([16, D], bf16, tag=f"etab9_{l}",
                                     name=f"etab9_{l}") for l in range(L)]
            w1t = [const_pool.tile([D, 2 * D], bf16, tag=f"w1t_{l}",
                                   name=f"w1t_{l}") for l in range(L)]
            b1t = [const_pool.tile([D, 2], f32, tag=f"b1t_{l}",
                                   name=f"b1t_{l}") for l in range(L)]
            w2s = [const_pool.tile([128, 2, 128], bf16, tag=f"w2s_{l}",
                                   name=f"w2s_{l}") for l in range(L)]
            b2t = [const_pool.tile([D, 1], f32, tag=f"b2t_{l}",
                                   name=f"b2t_{l}") for l in range(L)]
            for l in range(L):
                nc.sync.dma_start(etab9[l][:, :], etab9_in[l])
                nc.sync.dma_start(w1t[l][:, :], w1t_in[l])
                nc.sync.dma_start(b1t[l][:, :], b1t_in[l])
                nc.sync.dma_start(w2s[l][:, :, :], w2s_in[l])
                nc.sync.dma_start(b2t[l][:, :], b2t_in[l])

            def emit_ag1():
                nc.gpsimd.collective_compute(
                    "AllGather", mybir.AluOpType.bypass,
                    ins=[hownA[:, :]], outs=[hfullA[:, :]],
                    replica_groups=[list(range(cfg.NCORE))],
                )

            def emit_ag2():
                nc.gpsimd.collective_compute(
                    "AllGather", mybir.AluOpType.bypass,
                    ins=[hownB[:, :]], outs=[hfullB[:, :]],
                    replica_groups=[list(range(cfg.NCORE))],
                )

            # ---- layer-0 node embedding: h0 = onehot @ xemb6 ----
            AG1_TILE = HALF // 128 - 1      # last tile feeding hownA (48)
            for t in range(cfg.NTILE):
                cols = slice(t * 128, (t + 1) * 128)
                xoh_t = mlp_pool.tile([8, 128], bf16, tag="xoh_t")
                nc.sync.dma_start(xoh_t[:, :], xohT_in[:, cols])
                h0p = psA_pool.tile([128, D], f32, tag="tp")
                nc.tensor.matmul(h0p[:, :], xoh_t[:, :], xemb6[:, :],
                                 start=True, stop=True)
                nc.vector.tensor_copy(hown_sb[:, t, :], h0p[:, :])
                nc.sync.dma_start(hown_rows(t), hown_sb[:, t, :])
                if cfg.SPLIT_AG and t == AG1_TILE:
                    emit_ag1()
                if cfg.DEBUG_TAPS and "hown0" in cfg.TAPS:
                    dt0 = mlp_pool.tile([128, D], f32, tag="dbg")
                    nc.vector.tensor_copy(dt0[:, :], hown_sb[:, t, :])
                    nc.sync.dma_start(dbg_hown0[cols, :], dt0[:, :])
            if cfg.SPLIT_AG:
                emit_ag2()

            # ---- layers (window-major: AG halves overlap gather passes) ----
            # slot layout must match _cell_bases: block-major, window-minor
            bases = []
            off = 0
            for blk in cfg.blocks:
                row = []
                for w in range(cfg.NBLK):
                    row.append(off)
                    off += len(blk) * cfg.CAP
                bases.append(row)
            assert off == cfg.TOTSLOT

            for l in range(L):
                # SPLIT_AG: this layer's AllGathers were already emitted
                # during the previous layer's w3 pass (or after layer 0's
                # embedding), so ncfw overlaps the remaining gather preps.
                if not cfg.SPLIT_AG:
                    nc.gpsimd.collective_compute(
                        "AllGather", mybir.AluOpType.bypass,
                        ins=[hown_d[:, :]], outs=[hfull[:, :]],
                        replica_groups=[list(range(cfg.NCORE))],
                    )
                # zero the overflow accumulator
                r0 = 0
                while r0 < cfg.NPAD:
                    rows = min(512, cfg.NPAD - r0)
                    view = ovf_d[r0:r0 + rows, :].rearrange(
                        "(i o) d -> i (o d)", i=128)
                    nc.sync.dma_start(view, zerob[:, :, :].rearrange(
                        "p a b -> p (a b)")[:, : rows * D // 128])
                    r0 += rows
                if cfg.DEBUG_TAPS and l == 0 and "hfA" in cfg.TAPS:
                    for i in range(cfg.NCORE * HALF // 128):
                        rows = slice(i * 128, (i + 1) * 128)
                        db = mlp_pool.tile([128, D], bf16, tag="dbgb")
                        nc.sync.dma_start(db[:, :], hfullA[rows, :])
                        df = mlp_pool.tile([128, D], f32, tag="dbg")
                        nc.vector.tensor_copy(df[:, :], db[:, :])
                        nc.sync.dma_start(dbg_hfA[rows, :], df[:, :])
                for w in range(cfg.NBLK):
                    if cfg.SPLIT_AG:
                        src = hfullA if w < 2 else hfullB
                        woff = (w % 2) * cfg.WIN
                    else:
                        src = hfull
                        woff = w * cfg.WIN
                    # overflow edges of this window: gather then
                    # scatter-add into the ovf accumulator (sub-calls
                    # have distinct dst rows; -1 tails are skipped)
                    ogb = gather_pool.tile([128, cfg.OWIN // 128, D], bf16,
                                           tag="og")
                    oc = slice(w * cfg.OWIN // 16, (w + 1) * cfg.OWIN // 16)
                    nc.gpsimd.dma_gather(
                        ogb[:, :, :], src[woff:woff + cfg.WIN, :],
                        gidxo_t[:, oc], cfg.OWIN, cfg.OWIN, D,
                        single_packet=False, queue_num=0)
                    ooff = 0
                    for cap in cfg.OCAPS:
                        isl = slice((w * cfg.OWIN + ooff) // 16,
                                    (w * cfg.OWIN + ooff + cap) // 16)
                        nc.gpsimd.dma_scatter_add(
                            ovf_d[:, :],
                            ogb[:, ooff // 128:(ooff + cap) // 128, :],
                            sidxo_t[:, isl], cap, cap, D, queue_num=0)
                        ooff += cap
                    for bi, blk in enumerate(cfg.blocks):
                        nt = len(blk)
                        ng_call = nt * GPC
                        nidx = nt * cfg.CAP
                        base = bases[bi][w]
                        gbuf = gather_pool.tile([128, cfg.TPB * GPC, D], bf16,
                                                tag="g")
                        ic = slice(base // 16, (base + nidx) // 16)
                        nc.gpsimd.dma_gather(
                            gbuf[:, 0:ng_call, :],
                            src[woff:woff + cfg.WIN, :],
                            gidx_t[:, ic], nidx, nidx, D,
                            single_packet=False, queue_num=0)
                        sel_t = sel_pool.tile([128, cfg.TPB * GPC, 128], fp8,
                                              tag="sel")
                        gsl = slice(base // 128, (base + nidx) // 128)
                        nc.sync.dma_start(sel_t[:, 0:ng_call, :],
                                          selT_in[:, gsl, :])
                        agg = agg_pool.tile([128, cfg.TPB, 128], f32, tag="agg")
                        if w == 0:
                            # bank-filling first mm: self-loop identity
                            nc.tensor.matmul(
                                agg[:, 0:nt, :], identb[:, :],
                                hown_sb[:, blk[0]:blk[0] + nt, :],
                                start=True, stop=False, skip_group_check=True)
                            for i, t in enumerate(blk):
                                nc.tensor.matmul(
                                    agg[:, i, :],
                                    cntT[:, t * 128:(t + 1) * 128],
                                    etab9[l][:, :],
                                    start=False, stop=False,
                                    skip_group_check=True)
                        else:
                            # bank-clearing first mm (zero rhs)
                            nc.tensor.matmul(
                                agg[:, 0:nt, :], identb[:, :],
                                zerob[:, 0:nt, :],
                                start=True, stop=False, skip_group_check=True)
                        for g in range(ng_call):
                            last = (w != cfg.NBLK - 1) and (g == ng_call - 1)
                            nc.tensor.matmul(
                                agg[:, g // GPC, :], sel_t[:, g, :],
                                gbuf[:, g, :],
                                start=False, stop=last, skip_group_check=True)
                        if w == cfg.NBLK - 1:
                            # merge the overflow accumulator (one identity
                            # matmul per tile, still into the same bank)
                            for i, t in enumerate(blk):
                                ovt = mlp_pool.tile([128, D], bf16, tag="ovt")
                                nc.sync.dma_start(
                                    ovt[:, :],
                                    ovf_d[t * 128:(t + 1) * 128, :])
                                nc.tensor.matmul(
                                    agg[:, i, :], identb[:, :], ovt[:, :],
                                    start=False, stop=(i == nt - 1),
                                    skip_group_check=True)
                        # accumulate into SBUF
                        cols = slice(blk[0], blk[0] + nt)
                        if w == 0:
                            nc.vector.tensor_copy(agg_sb[:, cols, :],
                                                  agg[:, 0:nt, :])
                        else:
                            nc.vector.tensor_add(agg_sb[:, cols, :],
                                                 agg_sb[:, cols, :],
                                                 agg[:, 0:nt, :])

                        if w == cfg.NBLK - 1:
                            if cfg.DEBUG_TAPS and l == 0 and "agg" in cfg.TAPS:
                                for t in blk:
                                    rows = slice(t * 128, (t + 1) * 128)
                                    da = mlp_pool.tile([128, D], f32,
                                                       tag="dbg")
                                    nc.vector.tensor_copy(da[:, :],
                                                          agg_sb[:, t, :])
                                    nc.sync.dma_start(dbg_agg[rows, :],
                                                      da[:, :])
                            # ---- MLP per tile of this block ----
                            for i, t in enumerate(blk):
                                tp = psA_pool.tile([128, D], f32, tag="tp")
                                nc.tensor.transpose(tp[:, :], agg_sb[:, t, :],
                                                    identf[:, :])
                                tS = mlp_pool.tile([128, D], bf16, tag="tS")
                                nc.vector.tensor_copy(tS[:, :], tp[:, :])
                                # mm1 + relu + b1
                                hm = psB_pool.tile([128, 2, 128], f32, tag="hm")
                                hmS = mlp_pool.tile([128, 2, 128], bf16,
                                                    tag="hmS")
                                for j in range(2):
                                    nc.tensor.matmul(
                                        hm[:, j, :],
                                        w1t[l][:, j * 128:(j + 1) * 128],
                                        tS[:, :], start=True, stop=True)
                                    nc.scalar.activation(
                                        hmS[:, j, :], hm[:, j, :], relu,
                                        bias=b1t[l][:, j:j + 1])
                                # mm2 accumulate + bias (+relu if not last)
                                h2p = psA_pool.tile([128, D], f32, tag="h2p")
                                for j in range(2):
                                    nc.tensor.matmul(
                                        h2p[:, :], w2s[l][:, j, :],
                                        hmS[:, j, :],
                                        start=(j == 0), stop=(j == 1))
                                if l < L - 1:
                                    h2S = mlp_pool.tile([128, D], bf16,
                                                        tag="h2S")
                                    nc.scalar.activation(
                                        h2S[:, :], h2p[:, :], relu,
                                        bias=b2t[l][:, 0:1])
                                    op = psA_pool.tile([128, D], bf16,
                                                       tag="opb")
                                    nc.tensor.transpose(op[:, :], h2S[:, :],
                                                        identb[:, :])
                                    nc.vector.tensor_copy(hown_sb[:, t, :],
                                                          op[:, :])
                                    nc.sync.dma_start(
                                        hown_rows(t), hown_sb[:, t, :])
                                    if (cfg.DEBUG_TAPS
                                            and l == 0
                                            and "hown1" in cfg.TAPS):
                                        dh = mlp_pool.tile([128, D], f32,
                                                           tag="dbg")
                                        nc.vector.tensor_copy(
                                            dh[:, :], hown_sb[:, t, :])
                                        nc.sync.dma_start(
                                            dbg_hown1[t * 128:(t + 1) * 128,
                                                      :], dh[:, :])
                                else:
                                    h2S = mlp_pool.tile([128, D], f32,
                                                        tag="h2Sf")
                                    nc.vector.tensor_scalar_add(
                                        h2S[:, :], h2p[:, :], b2t[l][:, 0:1])
                                    op = psA_pool.tile([128, D], f32,
                                                       tag="opf")
                                    nc.tensor.transpose(op[:, :], h2S[:, :],
                                                        identf[:, :])
                                    oS = mlp_pool.tile([128, D], f32,
                                                       tag="oSf")
                                    nc.vector.tensor_copy(oS[:, :], op[:, :])
                                    nc.sync.dma_start(
                                        out_ext[t * 128:(t + 1) * 128, :],
                                        oS[:, :])
                            # emit next layer's AllGathers as soon as their
                            # input halves are complete, so ncfw overlaps
                            # the remaining w3 gather preps
                            if cfg.SPLIT_AG and l < L - 1:
                                if AG1_TILE in blk:
                                    emit_ag1()
                                if bi == len(cfg.blocks) - 1:
                                    emit_ag2()

    nc.finalize()
    return nc


_CACHE = {}


def _get_program(cfg):
    key = (cfg.N, cfg.CAP, cfg.TPB)
    if key not in _CACHE:
        _CACHE[key] = _build_program(cfg)
    return _CACHE[key]


def build_in_maps(cfg, inputs):
    params = _fold_params(
        cfg, inputs["x_emb"], inputs["etab"], inputs["w1"], inputs["b1"],
        inputs["w2"], inputs["b2"], inputs["gamma"], inputs["beta"],
        inputs["bn_mean"], inputs["bn_var"])
    per_core = _prepare_inputs(cfg, inputs["x"], inputs["edge_index"],
                               inputs["edge_attr"])
    in_maps = []
    for r in range(cfg.NCORE):
        m = dict(per_core[r])
        m.update({k: np.ascontiguousarray(v) for k, v in params.items()})
        in_maps.append(m)
    return in_maps


def kernel(**inputs) -> np.ndarray:
    cfg = CFG()
    nc = _get_program(cfg)
    in_maps = build_in_maps(cfg, inputs)
    res = run_bass_kernel_spmd(nc, in_maps, list(range(cfg.NCORE)))
    out = np.empty((cfg.N, cfg.D), np.float32)
    for r in range(cfg.NCORE):
        out[r * cfg.NOWN:(r + 1) * cfg.NOWN] = res.results[r]["out"][:cfg.NOWN]
    return out



# revision 6
# speedup vs baseline: 1.9076x; 1.9076x over previous
"""GIN-style 5-layer GNN message passing on 8 Trainium2 NeuronCores.

Strategy v4 (1D node-parallel, tensor-engine aggregation):
  - Nodes partitioned contiguously across 8 cores (12500 each, padded to
    12544 = 98*128). Edges owned by their dst core. The bottleneck is
    GPSIMD/SWDGE per-edge gather-descriptor generation (~8 ns/idx), so
    everything else is arranged to hide under it.
  - Per layer: two AllGathers (shard halves, bf16) build the full node
    table in DRAM; each half's ncfw work overlaps the previous/current
    gather passes.  dma_gather fetches h[src] per edge (256B rows) in
    dst-tile-sorted order, 2048 idxs per call, window-major passes.
  - Aggregation runs on the tensor engine, not the DMA scatter path:
    per 128-edge group (all edges of one dst tile), a one-hot fp8
    selection matrix contracts the gathered bf16 messages into a PSUM
    accumulator (out[dst, feat] += sel^T @ msg), window partials summed
    into an SBUF accumulator by the vector engine.  The self-loop term
    is the bank-filling first matmul (identity x h_own slab,
    start=True); edge embeddings fold into a counts matmul
    (counts^T @ etab).
  - Per-(tile, window) gather slots are capped at 512 (the cell mean is
    ~510); overflow edges (~3k/layer) go through a small classic
    dma_gather + dma_scatter_add pipeline into a DRAM side-accumulator,
    merged back by one identity matmul per tile.  Scatter sub-calls
    keep dst rows distinct (the DMA's RMW races on duplicates) and pad
    with trash rows >= NPAD (negative indices fault).
  - GIN MLP (D->2D->relu->D) + BatchNorm folded into the second linear,
    bf16 weights, computed per 128-node tile on the tensor engine.
"""
import sys
import numpy as np

sys.path.insert(0, "/opt/trn_rl_repo")

import ml_dtypes
import concourse.bass as bass
import concourse.bacc as bacc
import concourse.tile as tile
import concourse.masks as masks
from concourse import mybir
from concourse.bass_utils import run_bass_kernel_spmd


class CFG:
    DEBUG_TAPS = False
    TAPS = ()
    SPLIT_AG = True
    NQ = 4              # SWDGE queues (desc-gen parallelism)
    N = 100000          # total nodes
    D = 128             # feature dim
    L = 5               # layers
    NCORE = 8
    NOWN = 12500        # nodes per core
    NPAD = 12544        # padded nodes per core (98 * 128)
    NBLK = 4            # gather source windows (int16 idx limit)
    TPB = 4             # dst tiles per block
    CAP = 512           # slot capacity per (tile, window) cell; 4 groups
    # overflow sub-call capacities per window (scatter-add path);
    # each sub-call holds edges with distinct dst nodes
    OCAPS = (1024, 512, 256, 128, 128)
    EPS = 1e-5

    @property
    def OWIN(self):     # overflow slots per window
        return sum(self.OCAPS)

    @property
    def WIN(self):      # rows per gather window in h_full space
        return 2 * self.NPAD

    @property
    def NTILE(self):
        return self.NPAD // 128

    @property
    def HFULL(self):
        return self.NCORE * self.NPAD

    @property
    def GPC(self):      # groups per cell
        return self.CAP // 128

    @property
    def blocks(self):   # list of tile-index lists
        t = list(range(self.NTILE))
        return [t[i:i + self.TPB] for i in range(0, self.NTILE, self.TPB)]

    @property
    def TOTSLOT(self):
        return self.NTILE * self.NBLK * self.CAP

    @property
    def TOTGRP(self):
        return self.TOTSLOT // 128


def _f8(a):
    return np.asarray(a, np.float32).astype(ml_dtypes.float8_e4m3fn)


def _bf(a):
    return np.asarray(a, np.float32).astype(ml_dtypes.bfloat16)


def _fold_params(cfg, x_emb, etab, w1, b1, w2, b2, gamma, beta, bn_mean, bn_var):
    """Host-side parameter folding. Returns replicated device param arrays."""
    D, L = cfg.D, cfg.L
    x_emb = np.asarray(x_emb, np.float64)
    etab = np.asarray(etab, np.float64)
    w1 = np.asarray(w1, np.float64)
    b1 = np.asarray(b1, np.float64)
    w2 = np.asarray(w2, np.float64)
    b2 = np.asarray(b2, np.float64)
    gamma = np.asarray(gamma, np.float64)
    beta = np.asarray(beta, np.float64)
    bn_mean = np.asarray(bn_mean, np.float64)
    bn_var = np.asarray(bn_var, np.float64)

    xemb6 = np.zeros((8, D), np.float64)
    xemb6[0:3] = x_emb[0:3]
    xemb6[3:6] = x_emb[120:123]

    etab9 = np.zeros((L, 16, D), np.float64)
    etab9[:, 0:9, :] = etab

    w1t = np.ascontiguousarray(np.transpose(w1, (0, 2, 1)))          # [L,D,2D]
    b1t = np.ascontiguousarray(
        b1.reshape(L, 2, D).transpose(0, 2, 1)).astype(np.float32)   # [L,D,2]

    s = gamma / np.sqrt(bn_var + cfg.EPS)          # [L, D]
    t = beta - bn_mean * s
    w2f = w2 * s[:, :, None]                       # [L, D, 2D] rows scaled
    b2f = b2 * s + t                               # [L, D]
    # stationary chunks: w2s[l, p, k, m] = w2f[l, m, k*128 + p]
    w2s = np.ascontiguousarray(
        np.transpose(w2f.reshape(L, D, 2, D), (0, 3, 2, 1)))         # [L,128,2,128]
    b2t = b2f.astype(np.float32).reshape(L, D, 1)
    return dict(xemb6=_bf(xemb6), etab9=_bf(etab9), w1t=_bf(w1t), b1t=b1t,
                w2s=_bf(w2s), b2t=b2t)


def _wrap16(a):
    """Element i -> [i % 16, i // 16], replicated to 128 partitions."""
    assert len(a) % 16 == 0
    w = a.reshape(-1, 16).T
    return np.ascontiguousarray(np.tile(w, (8, 1)))


def _cell_bases(cfg):
    """slot base for each (tile, window) cell, matching the call layout:
    for each block B, for each window w, tiles of B consecutively."""
    bases = np.zeros((cfg.NTILE, cfg.NBLK), np.int64)
    off = 0
    for blk in cfg.blocks:
        for w in range(cfg.NBLK):
            for t in blk:
                bases[t, w] = off
                off += cfg.CAP
    assert off == cfg.TOTSLOT
    return bases


def _schedule_core(cfg, src_g, dst_l):
    """Assign this core's edges (global src, local dst) to gather slots.

    Returns gidx [TOTSLOT] int16 (window-local gather idx; 0 for padding)
    and selT [128, TOTGRP, 128] fp8 one-hot matrices (zero rows for pads).
    """
    # hfull is split into two AllGather halves: A = rank-major concat of
    # each core's rows [0:HALF), B = rows [HALF:NPAD).  Window w in {0,1}
    # addresses A, {2,3} addresses B.
    HALF = cfg.NPAD // 2
    q = src_g // cfg.NOWN
    local = src_g - q * cfg.NOWN
    if cfg.SPLIT_AG:
        in_a = local < HALF
        halfrow = np.where(in_a, q * HALF + local, q * HALF + local - HALF)
        w = halfrow // cfg.WIN + np.where(in_a, 0, 2)
        widx = (halfrow % cfg.WIN).astype(np.int64)
    else:
        src_row = q * cfg.NPAD + local
        w = src_row // cfg.WIN
        widx = (src_row % cfg.WIN).astype(np.int64)
    assert widx.max() < 2 ** 15
    t = dst_l // 128
    j = dst_l % 128

    cell = t * cfg.NBLK + w
    order = np.argsort(cell, kind="stable")
    cell_s = cell[order]
    counts = np.bincount(cell_s, minlength=cfg.NTILE * cfg.NBLK)
    bases = _cell_bases(cfg).reshape(-1)
    cell_starts = np.zeros(cfg.NTILE * cfg.NBLK + 1, np.int64)
    np.cumsum(counts, out=cell_starts[1:])
    pos_in_cell = np.arange(len(cell_s)) - cell_starts[cell_s]
    in_main = pos_in_cell < cfg.CAP
    slot = bases[cell_s] + pos_in_cell

    gidx = np.zeros(cfg.TOTSLOT, np.int16)
    gidx[slot[in_main]] = widx[order][in_main].astype(np.int16)
    selT = np.zeros((128, cfg.TOTGRP, 128), ml_dtypes.float8_e4m3fn)
    selT[slot[in_main] % 128, slot[in_main] // 128,
         j[order][in_main]] = 1.0

    # ---- overflow edges -> per-window scatter-add sub-calls ----
    # slot layout: [window][sub-call][cap]; within each sub-call dst
    # nodes are distinct.  Unused slots scatter row 0 of the window into
    # trash rows >= NPAD (negative "ignored" indices fault on this HW).
    nslot = cfg.NBLK * cfg.OWIN
    gidxo = np.zeros(nslot, np.int16)
    sidxo = (cfg.NPAD + np.arange(nslot) % 1024).astype(np.int16)
    ov = ~in_main
    ov_w = (cell_s % cfg.NBLK)[ov]
    ov_widx = widx[order][ov]
    ov_dst = dst_l[order][ov]
    for wv in range(cfg.NBLK):
        m = ov_w == wv
        dsts = ov_dst[m]
        wis = ov_widx[m]
        fill = [0] * len(cfg.OCAPS)
        used = [set() for _ in cfg.OCAPS]
        woff = wv * cfg.OWIN
        for dd, wi in zip(dsts.tolist(), wis.tolist()):
            for k, cap in enumerate(cfg.OCAPS):
                if fill[k] < cap and dd not in used[k]:
                    off = woff + sum(cfg.OCAPS[:k]) + fill[k]
                    gidxo[off] = wi
                    sidxo[off] = dd
                    used[k].add(dd)
                    fill[k] += 1
                    break
            else:
                raise AssertionError("overflow sub-calls exhausted")
    # gather pads must be valid indices (interior positions); the
    # matching scatter slots stay -1 so their values are never used
    gpad = gidxo < 0
    gidxo[gpad] = 0
    return gidx, selT, gidxo, sidxo


def _prepare_inputs(cfg, x, edge_index, edge_attr):
    """Per-core index/feature-side host prep."""
    x = np.asarray(x)
    src = np.asarray(edge_index[0], np.int64)
    dst = np.asarray(edge_index[1], np.int64)
    eb = np.asarray(edge_attr[:, 0], np.int64)
    ed = np.asarray(edge_attr[:, 1], np.int64)

    per_core = []
    owner = dst // cfg.NOWN
    for r in range(cfg.NCORE):
        m = owner == r
        dst_l = dst[m] - r * cfg.NOWN
        gidx, selT, gidxo, sidxo = _schedule_core(cfg, src[m], dst_l)

        # counts[j, n]: incoming (incl self-loop) bond-type/direction counts
        countsT = np.zeros((16, cfg.NPAD), np.float32)
        np.add.at(countsT, (eb[m], dst_l), 1.0)
        np.add.at(countsT, (6 + ed[m], dst_l), 1.0)
        loc = np.arange(cfg.NOWN)
        countsT[4, loc] += 1.0   # self-loop bond type 4
        countsT[6, loc] += 1.0   # self-loop direction 0

        xohT = np.zeros((8, cfg.NPAD), np.float32)
        xl = np.asarray(x[r * cfg.NOWN:(r + 1) * cfg.NOWN], np.int64)
        xohT[xl[:, 0], loc] = 1.0
        xohT[3 + xl[:, 1], loc] += 1.0

        per_core.append(dict(
            gidx=_wrap16(gidx), selT=selT,
            gidxo=_wrap16(gidxo), sidxo=_wrap16(sidxo),
            countsT=_bf(countsT), xohT=_bf(xohT),
        ))
    return per_core


def _build_program(cfg):
    nc = bacc.Bacc(None, target_bir_lowering=False, debug=True,
                   num_swdge_queues=cfg.NQ)
    f32, bf16, i16 = mybir.dt.float32, mybir.dt.bfloat16, mybir.dt.int16
    fp8 = mybir.dt.float8e4
    D, L = cfg.D, cfg.L
    GPC = cfg.GPC

    # I/O
    gidx_in = nc.dram_tensor("gidx", [128, cfg.TOTSLOT // 16], i16,
                             kind="ExternalInput")
    gidxo_in = nc.dram_tensor("gidxo", [128, cfg.NBLK * cfg.OWIN // 16], i16,
                              kind="ExternalInput")
    sidxo_in = nc.dram_tensor("sidxo", [128, cfg.NBLK * cfg.OWIN // 16], i16,
                              kind="ExternalInput")
    selT_in = nc.dram_tensor("selT", [128, cfg.TOTGRP, 128], fp8,
                             kind="ExternalInput")
    countsT_in = nc.dram_tensor("countsT", [16, cfg.NPAD], bf16,
                                kind="ExternalInput")
    xohT_in = nc.dram_tensor("xohT", [8, cfg.NPAD], bf16, kind="ExternalInput")
    xemb6_in = nc.dram_tensor("xemb6", [8, D], bf16, kind="ExternalInput")
    etab9_in = nc.dram_tensor("etab9", [L, 16, D], bf16, kind="ExternalInput")
    w1t_in = nc.dram_tensor("w1t", [L, D, 2 * D], bf16, kind="ExternalInput")
    b1t_in = nc.dram_tensor("b1t", [L, D, 2], f32, kind="ExternalInput")
    w2s_in = nc.dram_tensor("w2s", [L, 128, 2, 128], bf16, kind="ExternalInput")
    b2t_in = nc.dram_tensor("b2t", [L, D, 1], f32, kind="ExternalInput")
    out_ext = nc.dram_tensor("out", [cfg.NPAD, D], f32, kind="ExternalOutput")
    if cfg.DEBUG_TAPS:
        dbg_hown0 = nc.dram_tensor("dbg_hown0", [cfg.NPAD, D], f32,
                                   kind="ExternalOutput")
        dbg_hfA = nc.dram_tensor("dbg_hfA", [cfg.NCORE * (cfg.NPAD // 2), D],
                                 f32, kind="ExternalOutput")
        dbg_agg = nc.dram_tensor("dbg_agg", [cfg.NPAD, D], f32,
                                 kind="ExternalOutput")
        dbg_hown1 = nc.dram_tensor("dbg_hown1", [cfg.NPAD, D], f32,
                                   kind="ExternalOutput")

    # internal DRAM (shared across layers; layers are serial)
    HALF = cfg.NPAD // 2
    if cfg.SPLIT_AG:
        hownA = nc.dram_tensor("hownA", [HALF, D], bf16)
        hownB = nc.dram_tensor("hownB", [HALF, D], bf16)
        hfullA = nc.dram_tensor("hfullA", [cfg.NCORE * HALF, D], bf16,
                                addr_space="Shared")
        hfullB = nc.dram_tensor("hfullB", [cfg.NCORE * HALF, D], bf16,
                                addr_space="Shared")
        hown_d = None
    else:
        hown_d = nc.dram_tensor("hown", [cfg.NPAD, D], bf16)
        hfull = nc.dram_tensor("hfull", [cfg.HFULL, D], bf16,
                               addr_space="Shared")
        hfullA = hfullB = None

    ovf_d = nc.dram_tensor("ovf", [cfg.NPAD + 1024, D], bf16)

    def hown_rows(t):
        # DMA destination for the h rows of tile t (128 rows)
        r0 = t * 128
        if not cfg.SPLIT_AG:
            return hown_d[r0:r0 + 128, :]
        if r0 < HALF:
            return hownA[r0:r0 + 128, :]
        return hownB[r0 - HALF:r0 - HALF + 128, :]

    relu = mybir.ActivationFunctionType.Relu
    import itertools
    qrr = itertools.cycle(range(cfg.NQ))

    with tile.TileContext(nc) as tc:
        with (
            tc.tile_pool(name="const", bufs=1) as const_pool,
            tc.tile_pool(name="gather", bufs=4) as gather_pool,
            tc.tile_pool(name="sel", bufs=4) as sel_pool,
            tc.tile_pool(name="mlp", bufs=3) as mlp_pool,
            tc.tile_pool(name="aggp", bufs=2, space="PSUM") as agg_pool,
            tc.tile_pool(name="psA", bufs=1, space="PSUM") as psA_pool,
            tc.tile_pool(name="psB", bufs=2, space="PSUM") as psB_pool,
        ):
            # ---- resident constants ----
            identf = const_pool.tile([128, 128], f32, tag="identf")
            masks.make_identity(nc, identf[:, :])
            identb = const_pool.tile([128, 128], bf16, tag="identb")
            nc.vector.tensor_copy(identb[:, :], identf[:, :])
            zerob = const_pool.tile([128, cfg.TPB, 128], bf16, tag="zerob")
            nc.gpsimd.memset(zerob[:, :, :], 0.0)
            agg_sb = const_pool.tile([128, cfg.NTILE, 128], f32, tag="agg_sb")
            gidx_t = const_pool.tile([128, cfg.TOTSLOT // 16], i16, tag="gidx")
            nc.sync.dma_start(gidx_t[:, :], gidx_in[:, :])
            gidxo_t = const_pool.tile([128, cfg.NBLK * cfg.OWIN // 16], i16,
                                      tag="gidxo")
            nc.sync.dma_start(gidxo_t[:, :], gidxo_in[:, :])
            sidxo_t = const_pool.tile([128, cfg.NBLK * cfg.OWIN // 16], i16,
                                      tag="sidxo")
            nc.sync.dma_start(sidxo_t[:, :], sidxo_in[:, :])
            cntT = const_pool.tile([16, cfg.NPAD], bf16, tag="cntT")
            nc.sync.dma_start(cntT[:, :], countsT_in[:, :])
            hown_sb = const_pool.tile([128, cfg.NTILE, 128], bf16, tag="hown_sb")
            xemb6 = const_pool.tile([8, D], bf16, tag="xemb6")
            nc.sync.dma_start(xemb6[:, :], xemb6_in[:, :])
            etab9 = [const_pool.tile([16, D], bf16, tag=f"etab9_{l}",
                                     name=f"etab9_{l}") for l in range(L)]
            w1t = [const_pool.tile([D, 2 * D], bf16, tag=f"w1t_{l}",
                                   name=f"w1t_{l}") for l in range(L)]
            b1t = [const_pool.tile([D, 2], f32, tag=f"b1t_{l}",
                                   name=f"b1t_{l}") for l in range(L)]
            w2s = [const_pool.tile([128, 2, 128], bf16, tag=f"w2s_{l}",
                                   name=f"w2s_{l}") for l in range(L)]
            b2t = [const_pool.tile([D, 1], f32, tag=f"b2t_{l}",
                                   name=f"b2t_{l}") for l in range(L)]
            for l in range(L):
                nc.sync.dma_start(etab9[l][:, :], etab9_in[l])
                nc.sync.dma_start(w1t[l][:, :], w1t_in[l])
                nc.sync.dma_start(b1t[l][:, :], b1t_in[l])
                nc.sync.dma_start(w2s[l][:, :, :], w2s_in[l])
                nc.sync.dma_start(b2t[l][:, :], b2t_in[l])

            def emit_ag1():
                nc.gpsimd.collective_compute(
                    "AllGather", mybir.AluOpType.bypass,
                    ins=[hownA[:, :]], outs=[hfullA[:, :]],
                    replica_groups=[list(range(cfg.NCORE))],
                )

            def emit_ag2():
                nc.gpsimd.collective_compute(
                    "AllGather", mybir.AluOpType.bypass,
                    ins=[hownB[:, :]], outs=[hfullB[:, :]],
                    replica_groups=[list(range(cfg.NCORE))],
                )

            # ---- layer-0 node embedding: h0 = onehot @ xemb6 ----
            AG1_TILE = HALF // 128 - 1      # last tile feeding hownA (48)
            for t in range(cfg.NTILE):
                cols = slice(t * 128, (t + 1) * 128)
                xoh_t = mlp_pool.tile([8, 128], bf16, tag="xoh_t")
                nc.sync.dma_start(xoh_t[:, :], xohT_in[:, cols])
                h0p = psA_pool.tile([128, D], f32, tag="tp")
                nc.tensor.matmul(h0p[:, :], xoh_t[:, :], xemb6[:, :],
                                 start=True, stop=True)
                nc.vector.tensor_copy(hown_sb[:, t, :], h0p[:, :])
                nc.sync.dma_start(hown_rows(t), hown_sb[:, t, :])
                if cfg.SPLIT_AG and t == AG1_TILE:
                    emit_ag1()
                if cfg.DEBUG_TAPS and "hown0" in cfg.TAPS:
                    dt0 = mlp_pool.tile([128, D], f32, tag="dbg")
                    nc.vector.tensor_copy(dt0[:, :], hown_sb[:, t, :])
                    nc.sync.dma_start(dbg_hown0[cols, :], dt0[:, :])
            if cfg.SPLIT_AG:
                emit_ag2()

            # ---- layers (window-major: AG halves overlap gather passes) ----
            # slot layout must match _cell_bases: block-major, window-minor
            bases = []
            off = 0
            for blk in cfg.blocks:
                row = []
                for w in range(cfg.NBLK):
                    row.append(off)
                    off += len(blk) * cfg.CAP
                bases.append(row)
            assert off == cfg.TOTSLOT

            for l in range(L):
                # SPLIT_AG: this layer's AllGathers were already emitted
                # during the previous layer's w3 pass (or after layer 0's
                # embedding), so ncfw overlaps the remaining gather preps.
                if not cfg.SPLIT_AG:
                    nc.gpsimd.collective_compute(
                        "AllGather", mybir.AluOpType.bypass,
                        ins=[hown_d[:, :]], outs=[hfull[:, :]],
                        replica_groups=[list(range(cfg.NCORE))],
                    )
                # zero the overflow accumulator
                r0 = 0
                while r0 < cfg.NPAD:
                    rows = min(512, cfg.NPAD - r0)
                    view = ovf_d[r0:r0 + rows, :].rearrange(
                        "(i o) d -> i (o d)", i=128)
                    nc.sync.dma_start(view, zerob[:, :, :].rearrange(
                        "p a b -> p (a b)")[:, : rows * D // 128])
                    r0 += rows
                if cfg.DEBUG_TAPS and l == 0 and "hfA" in cfg.TAPS:
                    for i in range(cfg.NCORE * HALF // 128):
                        rows = slice(i * 128, (i + 1) * 128)
                        db = mlp_pool.tile([128, D], bf16, tag="dbgb")
                        nc.sync.dma_start(db[:, :], hfullA[rows, :])
                        df = mlp_pool.tile([128, D], f32, tag="dbg")
                        nc.vector.tensor_copy(df[:, :], db[:, :])
                        nc.sync.dma_start(dbg_hfA[rows, :], df[:, :])
                for w in range(cfg.NBLK):
                    if cfg.SPLIT_AG:
                        src = hfullA if w < 2 else hfullB
                        woff = (w % 2) * cfg.WIN
                    else:
                        src = hfull
                        woff = w * cfg.WIN
                    # overflow edges of this window: gather then
                    # scatter-add into the ovf accumulator (sub-calls
                    # have distinct dst rows; -1 tails are skipped)
                    ogb = gather_pool.tile([128, cfg.OWIN // 128, D], bf16,
                                           tag="og")
                    oc = slice(w * cfg.OWIN // 16, (w + 1) * cfg.OWIN // 16)
                    nc.gpsimd.dma_gather(
                        ogb[:, :, :], src[woff:woff + cfg.WIN, :],
                        gidxo_t[:, oc], cfg.OWIN, cfg.OWIN, D,
                        single_packet=False, queue_num=next(qrr))
                    ooff = 0
                    for cap in cfg.OCAPS:
                        isl = slice((w * cfg.OWIN + ooff) // 16,
                                    (w * cfg.OWIN + ooff + cap) // 16)
                        nc.gpsimd.dma_scatter_add(
                            ovf_d[:, :],
                            ogb[:, ooff // 128:(ooff + cap) // 128, :],
                            sidxo_t[:, isl], cap, cap, D, queue_num=0)
                        ooff += cap
                    for bi, blk in enumerate(cfg.blocks):
                        nt = len(blk)
                        ng_call = nt * GPC
                        nidx = nt * cfg.CAP
                        base = bases[bi][w]
                        gbuf = gather_pool.tile([128, cfg.TPB * GPC, D], bf16,
                                                tag="g")
                        ic = slice(base // 16, (base + nidx) // 16)
                        nc.gpsimd.dma_gather(
                            gbuf[:, 0:ng_call, :],
                            src[woff:woff + cfg.WIN, :],
                            gidx_t[:, ic], nidx, nidx, D,
                            single_packet=False, queue_num=next(qrr))
                        sel_t = sel_pool.tile([128, cfg.TPB * GPC, 128], fp8,
                                              tag="sel")
                        gsl = slice(base // 128, (base + nidx) // 128)
                        nc.sync.dma_start(sel_t[:, 0:ng_call, :],
                                          selT_in[:, gsl, :])
                        agg = agg_pool.tile([128, cfg.TPB, 128], f32, tag="agg")
                        if w == 0:
                            # bank-filling first mm: self-loop identity
                            nc.tensor.matmul(
                                agg[:, 0:nt, :], identb[:, :],
                                hown_sb[:, blk[0]:blk[0] + nt, :],
                                start=True, stop=False, skip_group_check=True)
                            for i, t in enumerate(blk):
                                nc.tensor.matmul(
                                    agg[:, i, :],
                                    cntT[:, t * 128:(t + 1) * 128],
                                    etab9[l][:, :],
                                    start=False, stop=False,
                                    skip_group_check=True)
                        else:
                            # bank-clearing first mm (zero rhs)
                            nc.tensor.matmul(
                                agg[:, 0:nt, :], identb[:, :],
                                zerob[:, 0:nt, :],
                                start=True, stop=False, skip_group_check=True)
                        for g in range(ng_call):
                            last = (w != cfg.NBLK - 1) and (g == ng_call - 1)
                            nc.tensor.matmul(
                                agg[:, g // GPC, :], sel_t[:, g, :],
                                gbuf[:, g, :],
                                start=False, stop=last, skip_group_check=True)
                        if w == cfg.NBLK - 1:
                            # merge the overflow accumulator (one identity
                            # matmul per tile, still into the same bank)
                            for i, t in enumerate(blk):
                                ovt = mlp_pool.tile([128, D], bf16, tag="ovt")
                                nc.sync.dma_start(
                                    ovt[:, :],
                                    ovf_d[t * 128:(t + 1) * 128, :])
                                nc.tensor.matmul(
                                    agg[:, i, :], identb[:, :], ovt[:, :],
                                    start=False, stop=(i == nt - 1),
                                    skip_group_check=True)
                        # accumulate into SBUF
                        cols = slice(blk[0], blk[0] + nt)
                        if w == 0:
                            nc.vector.tensor_copy(agg_sb[:, cols, :],
                                                  agg[:, 0:nt, :])
                        else:
                            nc.vector.tensor_add(agg_sb[:, cols, :],
                                                 agg_sb[:, cols, :],
                                                 agg[:, 0:nt, :])

                        if w == cfg.NBLK - 1:
                            if cfg.DEBUG_TAPS and l == 0 and "agg" in cfg.TAPS:
                                for t in blk:
                                    rows = slice(t * 128, (t + 1) * 128)
                                    da = mlp_pool.tile([128, D], f32,
                                                       tag="dbg")
                                    nc.vector.tensor_copy(da[:, :],
                                                          agg_sb[:, t, :])
                                    nc.sync.dma_start(dbg_agg[rows, :],
                                                      da[:, :])
                            # ---- MLP per tile of this block ----
                            for i, t in enumerate(blk):
                                tp = psA_pool.tile([128, D], f32, tag="tp")
                                nc.tensor.transpose(tp[:, :], agg_sb[:, t, :],
                                                    identf[:, :])
                                tS = mlp_pool.tile([128, D], bf16, tag="tS")
                                nc.vector.tensor_copy(tS[:, :], tp[:, :])
                                # mm1 + relu + b1
                                hm = psB_pool.tile([128, 2, 128], f32, tag="hm")
                                hmS = mlp_pool.tile([128, 2, 128], bf16,
                                                    tag="hmS")
                                for j in range(2):
                                    nc.tensor.matmul(
                                        hm[:, j, :],
                                        w1t[l][:, j * 128:(j + 1) * 128],
                                        tS[:, :], start=True, stop=True)
                                    nc.scalar.activation(
                                        hmS[:, j, :], hm[:, j, :], relu,
                                        bias=b1t[l][:, j:j + 1])
                                # mm2 accumulate + bias (+relu if not last)
                                h2p = psA_pool.tile([128, D], f32, tag="h2p")
                                for j in range(2):
                                    nc.tensor.matmul(
                                        h2p[:, :], w2s[l][:, j, :],
                                        hmS[:, j, :],
                                        start=(j == 0), stop=(j == 1))
                                if l < L - 1:
                                    h2S = mlp_pool.tile([128, D], bf16,
                                                        tag="h2S")
                                    nc.scalar.activation(
                                        h2S[:, :], h2p[:, :], relu,
                                        bias=b2t[l][:, 0:1])
                                    op = psA_pool.tile([128, D], bf16,
                                                       tag="opb")
                                    nc.tensor.transpose(op[:, :], h2S[:, :],
                                                        identb[:, :])
                                    nc.vector.tensor_copy(hown_sb[:, t, :],
                                                          op[:, :])
                                    nc.sync.dma_start(
                                        hown_rows(t), hown_sb[:, t, :])
                                    if (cfg.DEBUG_TAPS
                                            and l == 0
                                            and "hown1" in cfg.TAPS):
                                        dh = mlp_pool.tile([128, D], f32,
                                                           tag="dbg")
                                        nc.vector.tensor_copy(
                                            dh[:, :], hown_sb[:, t, :])
                                        nc.sync.dma_start(
                                            dbg_hown1[t * 128:(t + 1) * 128,
                                                      :], dh[:, :])
                                else:
                                    h2S = mlp_pool.tile([128, D], f32,
                                                        tag="h2Sf")
                                    nc.vector.tensor_scalar_add(
                                        h2S[:, :], h2p[:, :], b2t[l][:, 0:1])
                                    op = psA_pool.tile([128, D], f32,
                                                       tag="opf")
                                    nc.tensor.transpose(op[:, :], h2S[:, :],
                                                        identf[:, :])
                                    oS = mlp_pool.tile([128, D], f32,
                                                       tag="oSf")
                                    nc.vector.tensor_copy(oS[:, :], op[:, :])
                                    nc.sync.dma_start(
                                        out_ext[t * 128:(t + 1) * 128, :],
                                        oS[:, :])
                            # emit next layer's AllGathers as soon as their
                            # input halves are complete, so ncfw overlaps
                            # the remaining w3 gather preps
                            if cfg.SPLIT_AG and l < L - 1:
                                if AG1_TILE in blk:
                                    emit_ag1()
                                if bi == len(cfg.blocks) - 1:
                                    emit_ag2()

    nc.finalize()
    return nc


_CACHE = {}


def _get_program(cfg):
    key = (cfg.N, cfg.CAP, cfg.TPB)
    if key not in _CACHE:
        _CACHE[key] = _build_program(cfg)
    return _CACHE[key]


def build_in_maps(cfg, inputs):
    params = _fold_params(
        cfg, inputs["x_emb"], inputs["etab"], inputs["w1"], inputs["b1"],
        inputs["w2"], inputs["b2"], inputs["gamma"], inputs["beta"],
        inputs["bn_mean"], inputs["bn_var"])
    per_core = _prepare_inputs(cfg, inputs["x"], inputs["edge_index"],
                               inputs["edge_attr"])
    in_maps = []
    for r in range(cfg.NCORE):
        m = dict(per_core[r])
        m.update({k: np.ascontiguousarray(v) for k, v in params.items()})
        in_maps.append(m)
    return in_maps


def kernel(**inputs) -> np.ndarray:
    cfg = CFG()
    nc = _get_program(cfg)
    in_maps = build_in_maps(cfg, inputs)
    res = run_bass_kernel_spmd(nc, in_maps, list(range(cfg.NCORE)))
    out = np.empty((cfg.N, cfg.D), np.float32)
    for r in range(cfg.NCORE):
        out[r * cfg.NOWN:(r + 1) * cfg.NOWN] = res.results[r]["out"][:cfg.NOWN]
    return out

